# revision 30
# baseline (speedup 1.0000x reference)
"""Trainium2 Bass kernel for nn_IntraInterLoss (N=65536, D=768, 8 cores).

Math: with n_i = emb_i/||emb_i||, m1 = [target==1], m0 = 1-m1:
  s1 = sum_i m1_i n_i      s0 = sum_i m0_i n_i
  c1 = sum m1              c0 = N - c1
  out = <s1,s0>/(c1 c0) - 0.4 <s1,s1>/c1^2 - 0.1 <s0,s0>/c0^2

Key identity: s1 = w1 @ emb with per-row scalar w1_i = m1_i/||emb_i||, so the
masked normalized sums are two rows of a [2 x 8192] @ [8192 x 768] matmul the
TensorEngine accumulates in PSUM. The device computes [s1, s_all] (lhsT
columns [w1, rs]); the host recovers s0 = s_all - s1 and the count c1 from
target directly. Per 128-row tile:
  - row sumsq: 3 of 8 tiles per group via ACT Square+accum (~1.2us each),
    5 via ONE DVE scalar_tensor_tensor op ((x*1)*x with accum_out,
    ~0.87us) — both self-contained into sumsq[:, j]: no bn postprocess,
    no GpSimd stage, so the sumsq->sqrt->recip chain only ever crosses
    ACT<->DVE and the in-order queues can't fall into cross-engine
    lockstep (the GpSimd 3-op post was the previous tail bottleneck)
  - rs = 1/sqrt(sumsq): ACT Sqrt + DVE reciprocal, batched per group/chunk
  - matmul psum[2,:] += [w1|rs].T @ emb_tile (fp32r: 1 PE cycle/row vs 4
    for plain fp32), PSUM-accumulated over all 64 tiles
Pipeline: 7 full 3MiB DMA groups (24KB per-partition runs) + a 4-tile and
two 2-tile tail chunks, all issued from Sync (a Scalar-issued first DMA
lands on a slow ring and delays every in-order consumer queue ~7us).
After the last bytes land only one 2-tile chain remains (~5us) instead
of an 8-tile backlog. 5-deep buffering keeps the 16 HW DMA engines at
the ~420GB/s per-core share. Things that measured SLOWER: finer tails
with single-tile chunks or delayed (software-pipelined) consumption —
both trigger the tile scheduler's DMA issue pacing, which stalls Sync
mid-stream; Rsqrt/tensor_tensor_reduce/gpsimd-stt/gpsimd-PSUM are
rejected or crash this runtime.
Data-parallel over 8 cores (8192 rows each); host sums the 8 partial
(s1, s_all) and does the final three 768-dim dots.
Measured: ~79.5us HW exec fast-mode (baseline 80.2); slow-clock runs
~90-93us occur randomly for any variant; rel err ~1.6e-5.
"""

import numpy as np
from contextlib import ExitStack

import concourse.bass as bass
import concourse.bacc as bacc
import concourse.tile as tile
from concourse import mybir
from concourse.bass_utils import run_bass_kernel_spmd

N, D = 65536, 768
N_CORES = 8
SHARD = N // N_CORES          # 8192 rows per core
P = 128                       # SBUF partitions
T = SHARD // P                # 64 tiles of [128, 768] per core
G = 8                         # tiles per DRAM layout block (24KB per run)

F32 = mybir.dt.float32
F32R = mybir.dt.float32r
AF = mybir.ActivationFunctionType

_nc_cache = None


def _build_nc(EMB_BUFS=5, SCR_BUFS=2):
    nc = bacc.Bacc("TRN2", target_bir_lowering=False, debug=False,
                   num_devices=N_CORES)
    # fp32r: same 32-bit floats, but tagged so the PE streams them at 1
    # cycle/row (vs 4 for plain fp32). The BIR verifier requires every
    # producer feeding an fp32r matmul to emit fp32r, so the whole
    # emb path (DRAM -> DMA -> SBUF tile) is declared fp32r and views are
    # bitcast back to f32 for the ACT/DVE sumsq reads.
    emb = nc.dram_tensor("emb", [SHARD, D], F32R, kind="ExternalInput").ap()
    # mask as uint8: 8KB instead of 32KB on the DMA engines that carry it
    tgt = nc.dram_tensor("tgt", [P, T], mybir.dt.uint8,
                         kind="ExternalInput").ap()
    out_s = nc.dram_tensor("out_s", [2, D], F32, kind="ExternalOutput").ap()

    # 7 full 8-tile groups (24KB runs) + a 4-tile chunk (12KB runs) and two
    # 2-tile chunks (6KB runs). (Single-tile tails trigger the tile
    # scheduler's DMA issue pacing and starve the stream.)
    SCHED = [(g * 8, 8) for g in range(7)] + [(56, 4), (60, 2), (62, 2)]

    with tile.TileContext(nc) as tc, ExitStack() as ctx:
        embp = ctx.enter_context(tc.tile_pool(name="embp", bufs=EMB_BUFS))
        # each tail chunk gets its own buffer: chunk DMAs never wait on an
        # earlier chunk's consumers (issue-pacing stalls otherwise)
        embc = ctx.enter_context(tc.tile_pool(name="embc", bufs=4))
        scr = ctx.enter_context(tc.tile_pool(name="scr", bufs=SCR_BUFS))
        sing = ctx.enter_context(tc.tile_pool(name="sing", bufs=1))
        psum = ctx.enter_context(tc.tile_pool(name="psum", bufs=1, space="PSUM"))

        tgt_sb = sing.tile([P, T], F32)
        tgt_u8 = sing.tile([P, T], mybir.dt.uint8)
        warm = sing.tile([P, 1], F32)
        warm2 = sing.tile([P, 1], F32)

        sumsq = sing.tile([P, T], F32)
        rs = sing.tile([P, T], F32)
        w_all = sing.tile([P, T, 2], F32R)   # col0 = m1/|x|, col1 = 1/|x|

        acc_a = psum.tile([2, 512], F32)
        acc_b = psum.tile([2, 256], F32)

        # Contiguous-per-partition layout: row = g*(P*G) + p*G + k, i.e.
        # partition p of group g holds rows [g*P*G + p*G, +G) as one
        # contiguous 24KB DRAM run (optimal HBM streaming). Tile (g,k) is
        # rows {p*G+k}, a permutation of the shard; the masked sums are
        # row-order invariant and tgt uses the same layout.
        emb_g = emb.rearrange("(g p k) d -> g p k d", p=P, k=G)

        def sumsq_act(et, t, j):
            # ACT path: sumsq[:, j] = sum(x^2) via Square+accum
            s = scr.tile([P, D], F32, tag="scr_act")
            nc.scalar.activation(out=s, in_=et[:, t, :].bitcast(F32),
                                 func=AF.Square,
                                 accum_out=sumsq[:, j:j + 1])

        def sumsq_stt(et, t, j):
            # DVE path: one scalar_tensor_tensor op, (x*1)*x with the
            # per-partition accumulator -> sum(x^2); self-contained.
            s = scr.tile([P, D], F32, tag="scr_stt")
            x = et[:, t, :].bitcast(F32)
            with nc.allow_low_precision(reason="f32 accum reduce; tol 2e-2"):
                nc.vector.scalar_tensor_tensor(
                    out=s, in0=x, scalar=1.0, in1=x,
                    op0=mybir.AluOpType.mult, op1=mybir.AluOpType.mult,
                    accum_out=sumsq[:, j:j + 1])

        def w_cols(cs):
            nc.scalar.activation(out=rs[:, cs], in_=sumsq[:, cs],
                                 func=AF.Sqrt)
            # col1 = 1/|x|; col0 = m1/|x|
            with nc.allow_low_precision(reason="fp32r out == fp32 minus low bits; tol 2e-2"):
                nc.vector.reciprocal(out=w_all[:, cs, 1], in_=rs[:, cs])
            nc.vector.tensor_mul(w_all[:, cs, 0], tgt_sb[:, cs],
                                 w_all[:, cs, 1].bitcast(F32))

        def mm_tile(et, t, j):
            lhsT = w_all[:, j, :]
            first = (j == 0)
            last = (j == T - 1)
            nc.tensor.matmul(acc_a[:, :], lhsT, et[:, t, 0:512],
                             start=first, stop=last,
                             skip_group_check=True)
            nc.tensor.matmul(acc_b[:, :], lhsT, et[:, t, 512:768],
                             start=first, stop=last,
                             skip_group_check=True)

        for idx, (s0, n) in enumerate(SCHED):
            pool = embp if n == 8 else embc
            et = pool.tile([P, n, D], F32R)
            g0, o = divmod(s0, G)
            src = emb_g[g0] if n == G else emb_g[g0][:, o:o + n, :]
            # All emb DMAs issue from Sync: a Scalar-issued first group goes
            # out on a low-priority ring that trickles until ~24us, and the
            # in-order compute queues (headed by group 0's ops) idle behind
            # it — the PE then can't finish its ~43us of matmuls under the
            # stream. Sync-issued group 0 lands at ~16us instead.
            nc.sync.dma_start(out=et, in_=src)
            if idx == 0:
                # issued after the first emb group so the big stream starts
                # ~0.7us earlier; tgt isn't needed until the first w compute
                nc.sync.dma_start(out=tgt_u8, in_=tgt)
                nc.vector.tensor_copy(out=tgt_sb, in_=tgt_u8)
                # Pre-warm the ACT table set: force the Square+Sqrt table
                # loads into the prologue.
                nc.vector.memset(warm, 1.0)
                nc.scalar.activation(out=warm2, in_=warm, func=AF.Square)
                nc.scalar.activation(out=warm2, in_=warm, func=AF.Sqrt)
                # Warm the output-DMA path: a queue's first use costs ~8us
                # extra latency; pay it here instead of after the last matmul.
                nc.sync.dma_start(out=out_s[0:1, 0:1], in_=warm2[0:1, 0:1])

            # Engine split: ACT's square+accum costs ~1.2us/tile vs DVE's
            # ~0.87us stt, so head groups give ACT 3 of 8 tiles (DVE 5) to
            # keep both ~4.5us/group busy inside the 7.4us stream period —
            # ACT must not carry queue lag into the tail. Tail chunks
            # alternate, and the final tile 63 goes to DVE (fastest chain).
            # One batched w-chain and the matmuls per group/chunk
            # (same-section consumption keeps DMA semaphore recycling
            # fast -> no issue-pacing stalls).
            for t in range(n):
                j = s0 + t
                on_act = (t in (1, 4, 7)) if n == 8 else (j % 2 == 0)
                (sumsq_act if on_act else sumsq_stt)(et, t, j)
            w_cols(slice(s0, s0 + n))
            for t in range(n):
                mm_tile(et, t, s0 + t)

        out_s_sb = sing.tile([2, D], F32)
        # parallel PSUM->SBUF drain: ACT takes the 512 block, DVE the 256
        nc.scalar.copy(out=out_s_sb[:, 0:512], in_=acc_a[:, :])
        nc.vector.tensor_copy(out=out_s_sb[:, 512:768], in_=acc_b[:, :])
        # issue from Sync: its DMA_DIRECT2D costs ~650ns vs ~1190ns on
        # Scalar, and Sync is idle here while Scalar just ran the copy
        nc.sync.dma_start(out=out_s, in_=out_s_sb)

    nc.compile()
    return nc


def _get_nc():
    global _nc_cache
    if _nc_cache is None:
        _nc_cache = _build_nc()
    return _nc_cache


def _make_in_maps(emb, target):
    emb = np.ascontiguousarray(np.asarray(emb), dtype=np.float32)
    tgt = np.asarray(target).astype(np.uint8)  # values in {0,1}: cast IS the mask
    in_maps = []
    for c in range(N_CORES):
        sh = slice(c * SHARD, (c + 1) * SHARD)
        tgt_t = np.ascontiguousarray(
            tgt[sh].reshape(T // G, P, G).transpose(1, 0, 2).reshape(P, T))
        in_maps.append({"emb": emb[sh], "tgt": tgt_t})
    return in_maps


def run(emb, target, trace=False):
    """Returns (result_scalar_f32, BassKernelResults)."""
    nc = _get_nc()
    target = np.asarray(target)
    in_maps = _make_in_maps(emb, target)
    br = run_bass_kernel_spmd(nc, in_maps, list(range(N_CORES)), trace=trace)
    s = np.zeros((2, D), dtype=np.float64)
    for r in br.results:
        s += r["out_s"].astype(np.float64)
    s1 = s[0]
    s0 = s[1] - s[0]            # device row1 is s_all = s1 + s0
    c1 = float((target == 1).sum())
    c0 = N - c1
    val = (s1 @ s0) / (c1 * c0) - 0.4 * (s1 @ s1) / (c1 * c1) \
        - 0.1 * (s0 @ s0) / (c0 * c0)
    return np.float32(val), br


def kernel(emb, target):
    return run(emb, target)[0]


# revision 31
# speedup vs baseline: 1.0111x; 1.0111x over previous
"""Trainium2 Bass kernel for nn_IntraInterLoss (N=65536, D=768, 8 cores).

Math: with n_i = emb_i/||emb_i||, m1 = [target==1], m0 = 1-m1:
  s1 = sum_i m1_i n_i      s0 = sum_i m0_i n_i
  c1 = sum m1              c0 = N - c1
  out = <s1,s0>/(c1 c0) - 0.4 <s1,s1>/c1^2 - 0.1 <s0,s0>/c0^2

Key identity: s1 = w1 @ emb with per-row scalar w1_i = m1_i/||emb_i||, so the
masked normalized sums are two rows of a [2 x 8192] @ [8192 x 768] matmul the
TensorEngine accumulates in PSUM. The device computes [s1, s_all] (lhsT
columns [w1, rs]); the host recovers s0 = s_all - s1 and the count c1 from
target directly. Per 128-row tile:
  - row sumsq: 3 of 8 tiles per group via ACT Square+accum (~1.2us each),
    5 via ONE DVE scalar_tensor_tensor op ((x*1)*x with accum_out,
    ~0.87us) — both self-contained into sumsq[:, j]: no bn postprocess,
    no GpSimd stage, so the sumsq->sqrt->recip chain only ever crosses
    ACT<->DVE and the in-order queues can't fall into cross-engine
    lockstep (the GpSimd 3-op post was the previous tail bottleneck)
  - rs = 1/sqrt(sumsq): ACT Sqrt + DVE reciprocal, batched per group/chunk
  - matmul psum[2,:] += [w1|rs].T @ emb_tile (fp32r: 1 PE cycle/row vs 4
    for plain fp32), PSUM-accumulated over all 64 tiles
Pipeline: 7 full 3MiB DMA groups (24KB per-partition runs) + a 4-tile and
two 2-tile tail chunks, all issued from Sync (a Scalar-issued first DMA
lands on a slow ring and delays every in-order consumer queue ~7us).
After the last bytes land only one 2-tile chain remains (~5us) instead
of an 8-tile backlog. 5-deep buffering keeps the 16 HW DMA engines at
the ~420GB/s per-core share. Things that measured SLOWER: finer tails
with single-tile chunks or delayed (software-pipelined) consumption —
both trigger the tile scheduler's DMA issue pacing, which stalls Sync
mid-stream; Rsqrt/tensor_tensor_reduce/gpsimd-stt/gpsimd-PSUM are
rejected or crash this runtime.
Data-parallel over 8 cores (8192 rows each); host sums the 8 partial
(s1, s_all) and does the final three 768-dim dots.
Measured: ~79.5us HW exec fast-mode (baseline 80.2); slow-clock runs
~90-93us occur randomly for any variant; rel err ~1.6e-5.
"""

import numpy as np
from contextlib import ExitStack

import concourse.bass as bass
import concourse.bacc as bacc
import concourse.tile as tile
from concourse import mybir
from concourse.bass_utils import run_bass_kernel_spmd

N, D = 65536, 768
N_CORES = 8
SHARD = N // N_CORES          # 8192 rows per core
P = 128                       # SBUF partitions
T = SHARD // P                # 64 tiles of [128, 768] per core
G = 8                         # tiles per DRAM layout block (24KB per run)

F32 = mybir.dt.float32
F32R = mybir.dt.float32r
AF = mybir.ActivationFunctionType

_nc_cache = None


def _build_nc(EMB_BUFS=5, SCR_BUFS=2):
    nc = bacc.Bacc("TRN2", target_bir_lowering=False, debug=False,
                   num_devices=N_CORES)
    # fp32r: same 32-bit floats, but tagged so the PE streams them at 1
    # cycle/row (vs 4 for plain fp32). The BIR verifier requires every
    # producer feeding an fp32r matmul to emit fp32r, so the whole
    # emb path (DRAM -> DMA -> SBUF tile) is declared fp32r and views are
    # bitcast back to f32 for the ACT/DVE sumsq reads.
    emb = nc.dram_tensor("emb", [SHARD, D], F32R, kind="ExternalInput").ap()
    # mask as uint8: 8KB instead of 32KB on the DMA engines that carry it
    tgt = nc.dram_tensor("tgt", [P, T], mybir.dt.uint8,
                         kind="ExternalInput").ap()
    out_s = nc.dram_tensor("out_s", [2, D], F32, kind="ExternalOutput").ap()

    # 7 full 8-tile groups (24KB runs) + a 4-tile chunk, a 2-tile chunk,
    # and single tiles 62/63: the singles arrive ~0.9us apart, so tile 62's
    # whole chain (and its matmuls) runs under tile 63's stream, and the
    # final chain is one DVE stt + sqrt + recip/mul + one tile's matmuls.
    SCHED = [(g * 8, 8) for g in range(7)] + [(56, 4), (60, 2), (62, 1), (63, 1)]

    with tile.TileContext(nc) as tc, ExitStack() as ctx:
        embp = ctx.enter_context(tc.tile_pool(name="embp", bufs=EMB_BUFS))
        # each tail chunk gets its own buffer: chunk DMAs never wait on an
        # earlier chunk's consumers (issue-pacing stalls otherwise)
        embc = ctx.enter_context(tc.tile_pool(name="embc", bufs=4))
        scr = ctx.enter_context(tc.tile_pool(name="scr", bufs=SCR_BUFS))
        sing = ctx.enter_context(tc.tile_pool(name="sing", bufs=1))
        psum = ctx.enter_context(tc.tile_pool(name="psum", bufs=1, space="PSUM"))

        tgt_sb = sing.tile([P, T], F32)
        tgt_u8 = sing.tile([P, T], mybir.dt.uint8)
        warm = sing.tile([P, 1], F32)
        warm2 = sing.tile([P, 1], F32)

        sumsq = sing.tile([P, T], F32)
        rs = sing.tile([P, T], F32)
        w_all = sing.tile([P, T, 2], F32R)   # col0 = m1/|x|, col1 = 1/|x|

        acc_a = psum.tile([2, 512], F32)
        acc_b = psum.tile([2, 256], F32)

        # Contiguous-per-partition layout: row = g*(P*G) + p*G + k, i.e.
        # partition p of group g holds rows [g*P*G + p*G, +G) as one
        # contiguous 24KB DRAM run (optimal HBM streaming). Tile (g,k) is
        # rows {p*G+k}, a permutation of the shard; the masked sums are
        # row-order invariant and tgt uses the same layout.
        emb_g = emb.rearrange("(g p k) d -> g p k d", p=P, k=G)

        def sumsq_act(et, t, j):
            # ACT path: sumsq[:, j] = sum(x^2) via Square+accum
            s = scr.tile([P, D], F32, tag="scr_act")
            nc.scalar.activation(out=s, in_=et[:, t, :].bitcast(F32),
                                 func=AF.Square,
                                 accum_out=sumsq[:, j:j + 1])

        def sumsq_stt(et, t, j):
            # DVE path: one scalar_tensor_tensor op, (x*1)*x with the
            # per-partition accumulator -> sum(x^2); self-contained.
            s = scr.tile([P, D], F32, tag="scr_stt")
            x = et[:, t, :].bitcast(F32)
            with nc.allow_low_precision(reason="f32 accum reduce; tol 2e-2"):
                nc.vector.scalar_tensor_tensor(
                    out=s, in0=x, scalar=1.0, in1=x,
                    op0=mybir.AluOpType.mult, op1=mybir.AluOpType.mult,
                    accum_out=sumsq[:, j:j + 1])

        def w_cols(cs):
            nc.scalar.activation(out=rs[:, cs], in_=sumsq[:, cs],
                                 func=AF.Sqrt)
            # col1 = 1/|x|; col0 = m1/|x|
            with nc.allow_low_precision(reason="fp32r out == fp32 minus low bits; tol 2e-2"):
                nc.vector.reciprocal(out=w_all[:, cs, 1], in_=rs[:, cs])
            nc.vector.tensor_mul(w_all[:, cs, 0], tgt_sb[:, cs],
                                 w_all[:, cs, 1].bitcast(F32))

        def mm_tile(et, t, j):
            lhsT = w_all[:, j, :]
            first = (j == 0)
            last = (j == T - 1)
            nc.tensor.matmul(acc_a[:, :], lhsT, et[:, t, 0:512],
                             start=first, stop=last,
                             skip_group_check=True)
            nc.tensor.matmul(acc_b[:, :], lhsT, et[:, t, 512:768],
                             start=first, stop=last,
                             skip_group_check=True)

        for idx, (s0, n) in enumerate(SCHED):
            pool = embp if n == 8 else embc
            et = pool.tile([P, n, D], F32R)
            g0, o = divmod(s0, G)
            src = emb_g[g0] if n == G else emb_g[g0][:, o:o + n, :]
            # All emb DMAs issue from Sync: a Scalar-issued first group goes
            # out on a low-priority ring that trickles until ~24us, and the
            # in-order compute queues (headed by group 0's ops) idle behind
            # it — the PE then can't finish its ~43us of matmuls under the
            # stream. Sync-issued group 0 lands at ~16us instead.
            nc.sync.dma_start(out=et, in_=src)
            if idx == 0:
                # issued after the first emb group so the big stream starts
                # ~0.7us earlier; tgt isn't needed until the first w compute
                nc.sync.dma_start(out=tgt_u8, in_=tgt)
                nc.vector.tensor_copy(out=tgt_sb, in_=tgt_u8)
                # Pre-warm the ACT table set: force the Square+Sqrt table
                # loads into the prologue.
                nc.vector.memset(warm, 1.0)
                nc.scalar.activation(out=warm2, in_=warm, func=AF.Square)
                nc.scalar.activation(out=warm2, in_=warm, func=AF.Sqrt)
                # Warm the output-DMA path: a queue's first use costs ~8us
                # extra latency; pay it here instead of after the last matmul.
                nc.sync.dma_start(out=out_s[0:1, 0:1], in_=warm2[0:1, 0:1])

            # Engine split: ACT's square+accum costs ~1.2us/tile vs DVE's
            # ~0.87us stt, so head groups give ACT 3 of 8 tiles (DVE 5) to
            # keep both ~4.5us/group busy inside the 7.4us stream period —
            # ACT must not carry queue lag into the tail. Tail chunks
            # alternate, and the final tile 63 goes to DVE (fastest chain).
            # One batched w-chain and the matmuls per group/chunk
            # (same-section consumption keeps DMA semaphore recycling
            # fast -> no issue-pacing stalls).
            for t in range(n):
                j = s0 + t
                on_act = (t in (1, 4, 7)) if n == 8 else (j % 2 == 0)
                (sumsq_act if on_act else sumsq_stt)(et, t, j)
            w_cols(slice(s0, s0 + n))
            for t in range(n):
                mm_tile(et, t, s0 + t)

        out_s_sb = sing.tile([2, D], F32)
        # parallel PSUM->SBUF drain: ACT takes the 512 block, DVE the 256
        nc.scalar.copy(out=out_s_sb[:, 0:512], in_=acc_a[:, :])
        nc.vector.tensor_copy(out=out_s_sb[:, 512:768], in_=acc_b[:, :])
        # issue from Sync: its DMA_DIRECT2D costs ~650ns vs ~1190ns on
        # Scalar, and Sync is idle here while Scalar just ran the copy
        nc.sync.dma_start(out=out_s, in_=out_s_sb)

    nc.compile()
    return nc


def _get_nc():
    global _nc_cache
    if _nc_cache is None:
        _nc_cache = _build_nc()
    return _nc_cache


def _make_in_maps(emb, target):
    emb = np.ascontiguousarray(np.asarray(emb), dtype=np.float32)
    tgt = np.asarray(target).astype(np.uint8)  # values in {0,1}: cast IS the mask
    in_maps = []
    for c in range(N_CORES):
        sh = slice(c * SHARD, (c + 1) * SHARD)
        tgt_t = np.ascontiguousarray(
            tgt[sh].reshape(T // G, P, G).transpose(1, 0, 2).reshape(P, T))
        in_maps.append({"emb": emb[sh], "tgt": tgt_t})
    return in_maps


def run(emb, target, trace=False):
    """Returns (result_scalar_f32, BassKernelResults)."""
    nc = _get_nc()
    target = np.asarray(target)
    in_maps = _make_in_maps(emb, target)
    br = run_bass_kernel_spmd(nc, in_maps, list(range(N_CORES)), trace=trace)
    s = np.zeros((2, D), dtype=np.float64)
    for r in br.results:
        s += r["out_s"].astype(np.float64)
    s1 = s[0]
    s0 = s[1] - s[0]            # device row1 is s_all = s1 + s0
    c1 = float((target == 1).sum())
    c0 = N - c1
    val = (s1 @ s0) / (c1 * c0) - 0.4 * (s1 @ s1) / (c1 * c1) \
        - 0.1 * (s0 @ s0) / (c0 * c0)
    return np.float32(val), br


def kernel(emb, target):
    return run(emb, target)[0]


# revision 34
# speedup vs baseline: 1.0478x; 1.0363x over previous
"""Trainium2 Bass kernel for nn_IntraInterLoss (N=65536, D=768, 8 cores).

Math: with n_i = emb_i/||emb_i||, m1 = [target==1], m0 = 1-m1:
  s1 = sum_i m1_i n_i      s0 = sum_i m0_i n_i
  c1 = sum m1              c0 = N - c1
  out = <s1,s0>/(c1 c0) - 0.4 <s1,s1>/c1^2 - 0.1 <s0,s0>/c0^2

Key identity: s1 = w1 @ emb with per-row scalar w1_i = m1_i/||emb_i||, so the
masked normalized sums are two rows of a [2 x 8192] @ [8192 x 768] matmul the
TensorEngine accumulates in PSUM. The device computes [s1, s_all] (lhsT
columns [w1, rs]); the host recovers s0 = s_all - s1 and the count c1 from
target directly. Per 128-row tile:
  - row sumsq: 3 of 8 tiles per group via ACT Square+accum (~1.2us each),
    5 via ONE DVE scalar_tensor_tensor op ((x*1)*x with accum_out,
    ~0.87us) — both self-contained into sumsq[:, j]: no bn postprocess,
    no GpSimd stage, so the sumsq->sqrt->recip chain only ever crosses
    ACT<->DVE and the in-order queues can't fall into cross-engine
    lockstep (the GpSimd 3-op post was the previous tail bottleneck)
  - rs = 1/sqrt(sumsq): ACT Sqrt + DVE reciprocal, batched per group/chunk
  - matmul psum[2,:] += [w1|rs].T @ emb_tile (fp32r: 1 PE cycle/row vs 4
    for plain fp32), PSUM-accumulated over all 64 tiles
Pipeline: 7 full 3MiB DMA groups (24KB per-partition runs) + a 4-tile and
two 2-tile tail chunks, all issued from Sync (a Scalar-issued first DMA
lands on a slow ring and delays every in-order consumer queue ~7us).
After the last bytes land only one 2-tile chain remains (~5us) instead
of an 8-tile backlog. 5-deep buffering keeps the 16 HW DMA engines at
the ~420GB/s per-core share. Things that measured SLOWER: finer tails
with single-tile chunks or delayed (software-pipelined) consumption —
both trigger the tile scheduler's DMA issue pacing, which stalls Sync
mid-stream; Rsqrt/tensor_tensor_reduce/gpsimd-stt/gpsimd-PSUM are
rejected or crash this runtime.
Data-parallel over 8 cores (8192 rows each); host sums the 8 partial
(s1, s_all) and does the final three 768-dim dots.
Measured: ~79.5us HW exec fast-mode (baseline 80.2); slow-clock runs
~90-93us occur randomly for any variant; rel err ~1.6e-5.
"""

import numpy as np
from contextlib import ExitStack

import concourse.bass as bass
import concourse.bacc as bacc
import concourse.tile as tile
from concourse import mybir
from concourse.bass_utils import run_bass_kernel_spmd

N, D = 65536, 768
N_CORES = 8
SHARD = N // N_CORES          # 8192 rows per core
P = 128                       # SBUF partitions
T = SHARD // P                # 64 tiles of [128, 768] per core
G = 8                         # tiles per DRAM layout block (24KB per run)

F32 = mybir.dt.float32
F32R = mybir.dt.float32r
AF = mybir.ActivationFunctionType

_nc_cache = None


def _build_nc(EMB_BUFS=5, SCR_BUFS=2):
    nc = bacc.Bacc("TRN2", target_bir_lowering=False, debug=False,
                   num_devices=N_CORES)
    # fp32r: same 32-bit floats, but tagged so the PE streams them at 1
    # cycle/row (vs 4 for plain fp32). The BIR verifier requires every
    # producer feeding an fp32r matmul to emit fp32r, so the whole
    # emb path (DRAM -> DMA -> SBUF tile) is declared fp32r and views are
    # bitcast back to f32 for the ACT/DVE sumsq reads.
    emb = nc.dram_tensor("emb", [SHARD, D], F32R, kind="ExternalInput").ap()
    # mask as uint8: 8KB instead of 32KB on the DMA engines that carry it
    tgt = nc.dram_tensor("tgt", [P, T], mybir.dt.uint8,
                         kind="ExternalInput").ap()
    out_s = nc.dram_tensor("out_s", [2, D], F32, kind="ExternalOutput").ap()

    # One 8-tile group (24KB runs, covers PE start latency), then 4-tile
    # chunks (12KB runs): each chunk's w-chain completes within its own
    # 3.7us stream period, so the PE is fed continuously instead of
    # stalling ~2.8us per 8-tile group waiting for batched w (16us of PE
    # idle became a 1.8us matmul backlog after the last bytes). Tail ends
    # with a 2-tile chunk and single tiles 62/63: the final chain is one
    # DVE stt + sqrt + recip/mul + one tile's matmuls.
    SCHED = ([(0, 8)] + [(8 + 4 * c, 4) for c in range(13)]
             + [(60, 2), (62, 1), (63, 1)])

    with tile.TileContext(nc) as tc, ExitStack() as ctx:
        embp = ctx.enter_context(tc.tile_pool(name="embp", bufs=1))
        # deep ring for the 4-tile chunks: ~15MB in flight so chunk DMAs
        # never wait on consumers (issue-pacing stalls otherwise); the
        # 2-tile chunk and the singles get their own buffers
        emb4 = ctx.enter_context(tc.tile_pool(name="emb4", bufs=10))
        embc = ctx.enter_context(tc.tile_pool(name="embc", bufs=4))
        scr = ctx.enter_context(tc.tile_pool(name="scr", bufs=SCR_BUFS))
        sing = ctx.enter_context(tc.tile_pool(name="sing", bufs=1))
        psum = ctx.enter_context(tc.tile_pool(name="psum", bufs=1, space="PSUM"))

        tgt_sb = sing.tile([P, T], F32)
        tgt_u8 = sing.tile([P, T], mybir.dt.uint8)
        warm = sing.tile([P, 1], F32)
        warm2 = sing.tile([P, 1], F32)

        sumsq = sing.tile([P, T], F32)
        rs = sing.tile([P, T], F32)
        w_all = sing.tile([P, T, 2], F32R)   # col0 = m1/|x|, col1 = 1/|x|

        acc_a = psum.tile([2, 512], F32)
        acc_b = psum.tile([2, 256], F32)

        # Contiguous-per-partition layout: row = g*(P*G) + p*G + k, i.e.
        # partition p of group g holds rows [g*P*G + p*G, +G) as one
        # contiguous 24KB DRAM run (optimal HBM streaming). Tile (g,k) is
        # rows {p*G+k}, a permutation of the shard; the masked sums are
        # row-order invariant and tgt uses the same layout.
        emb_g = emb.rearrange("(g p k) d -> g p k d", p=P, k=G)

        def sumsq_act(et, t, j):
            # ACT path: sumsq[:, j] = sum(x^2) via Square+accum
            s = scr.tile([P, D], F32, tag="scr_act")
            nc.scalar.activation(out=s, in_=et[:, t, :].bitcast(F32),
                                 func=AF.Square,
                                 accum_out=sumsq[:, j:j + 1])

        def sumsq_stt(et, t, j):
            # DVE path: one scalar_tensor_tensor op, (x*1)*x with the
            # per-partition accumulator -> sum(x^2); self-contained.
            s = scr.tile([P, D], F32, tag="scr_stt")
            x = et[:, t, :].bitcast(F32)
            with nc.allow_low_precision(reason="f32 accum reduce; tol 2e-2"):
                nc.vector.scalar_tensor_tensor(
                    out=s, in0=x, scalar=1.0, in1=x,
                    op0=mybir.AluOpType.mult, op1=mybir.AluOpType.mult,
                    accum_out=sumsq[:, j:j + 1])

        def w_cols(cs):
            nc.scalar.activation(out=rs[:, cs], in_=sumsq[:, cs],
                                 func=AF.Sqrt)
            # col1 = 1/|x|; col0 = m1/|x|
            with nc.allow_low_precision(reason="fp32r out == fp32 minus low bits; tol 2e-2"):
                nc.vector.reciprocal(out=w_all[:, cs, 1], in_=rs[:, cs])
            nc.vector.tensor_mul(w_all[:, cs, 0], tgt_sb[:, cs],
                                 w_all[:, cs, 1].bitcast(F32))

        def mm_tile(et, t, j):
            lhsT = w_all[:, j, :]
            first = (j == 0)
            last = (j == T - 1)
            nc.tensor.matmul(acc_a[:, :], lhsT, et[:, t, 0:512],
                             start=first, stop=last,
                             skip_group_check=True)
            nc.tensor.matmul(acc_b[:, :], lhsT, et[:, t, 512:768],
                             start=first, stop=last,
                             skip_group_check=True)

        for idx, (s0, n) in enumerate(SCHED):
            pool = embp if n == 8 else (emb4 if n == 4 else embc)
            et = pool.tile([P, n, D], F32R)
            g0, o = divmod(s0, G)
            src = emb_g[g0] if n == G else emb_g[g0][:, o:o + n, :]
            # All emb DMAs issue from Sync: a Scalar-issued first group goes
            # out on a low-priority ring that trickles until ~24us, and the
            # in-order compute queues (headed by group 0's ops) idle behind
            # it — the PE then can't finish its ~43us of matmuls under the
            # stream. Sync-issued group 0 lands at ~16us instead.
            nc.sync.dma_start(out=et, in_=src)
            if idx == 0:
                # issued after the first emb group so the big stream starts
                # ~0.7us earlier; tgt isn't needed until the first w compute
                nc.sync.dma_start(out=tgt_u8, in_=tgt)
                nc.vector.tensor_copy(out=tgt_sb, in_=tgt_u8)
                # Pre-warm the ACT table set: force the Square+Sqrt table
                # loads into the prologue.
                nc.vector.memset(warm, 1.0)
                nc.scalar.activation(out=warm2, in_=warm, func=AF.Square)
                nc.scalar.activation(out=warm2, in_=warm, func=AF.Sqrt)
                # Warm the output-DMA path: a queue's first use costs ~8us
                # extra latency; pay it here instead of after the last matmul.
                nc.sync.dma_start(out=out_s[0:1, 0:1], in_=warm2[0:1, 0:1])

            # Engine split: ACT's square+accum costs ~1.2us/tile vs DVE's
            # ~0.87us stt, so head groups give ACT 3 of 8 tiles (DVE 5) to
            # keep both ~4.5us/group busy inside the 7.4us stream period —
            # ACT must not carry queue lag into the tail. Tail chunks
            # alternate, and the final tile 63 goes to DVE (fastest chain).
            # One batched w-chain and the matmuls per group/chunk
            # (same-section consumption keeps DMA semaphore recycling
            # fast -> no issue-pacing stalls).
            for t in range(n):
                j = s0 + t
                on_act = (t in (1, 4, 7)) if n == 8 else (j % 2 == 0)
                (sumsq_act if on_act else sumsq_stt)(et, t, j)
            w_cols(slice(s0, s0 + n))
            for t in range(n):
                mm_tile(et, t, s0 + t)

        out_s_sb = sing.tile([2, D], F32)
        # parallel PSUM->SBUF drain: ACT takes the 512 block, DVE the 256
        nc.scalar.copy(out=out_s_sb[:, 0:512], in_=acc_a[:, :])
        nc.vector.tensor_copy(out=out_s_sb[:, 512:768], in_=acc_b[:, :])
        # issue from Sync: its DMA_DIRECT2D costs ~650ns vs ~1190ns on
        # Scalar, and Sync is idle here while Scalar just ran the copy
        nc.sync.dma_start(out=out_s, in_=out_s_sb)

    nc.compile()
    return nc


def _get_nc():
    global _nc_cache
    if _nc_cache is None:
        _nc_cache = _build_nc()
    return _nc_cache


def _make_in_maps(emb, target):
    emb = np.ascontiguousarray(np.asarray(emb), dtype=np.float32)
    tgt = np.asarray(target).astype(np.uint8)  # values in {0,1}: cast IS the mask
    in_maps = []
    for c in range(N_CORES):
        sh = slice(c * SHARD, (c + 1) * SHARD)
        tgt_t = np.ascontiguousarray(
            tgt[sh].reshape(T // G, P, G).transpose(1, 0, 2).reshape(P, T))
        in_maps.append({"emb": emb[sh], "tgt": tgt_t})
    return in_maps


def run(emb, target, trace=False):
    """Returns (result_scalar_f32, BassKernelResults)."""
    nc = _get_nc()
    target = np.asarray(target)
    in_maps = _make_in_maps(emb, target)
    br = run_bass_kernel_spmd(nc, in_maps, list(range(N_CORES)), trace=trace)
    s = np.zeros((2, D), dtype=np.float64)
    for r in br.results:
        s += r["out_s"].astype(np.float64)
    s1 = s[0]
    s0 = s[1] - s[0]            # device row1 is s_all = s1 + s0
    c1 = float((target == 1).sum())
    c0 = N - c1
    val = (s1 @ s0) / (c1 * c0) - 0.4 * (s1 @ s1) / (c1 * c1) \
        - 0.1 * (s0 @ s0) / (c0 * c0)
    return np.float32(val), br


def kernel(emb, target):
    return run(emb, target)[0]
